# revision 11
# baseline (speedup 1.0000x reference)
"""MoE top-8 routing kernel for Trainium2 (8 NeuronCores, data-parallel).

Computes, for each of 262144 tokens with 128 expert logits:
  values, indices = top_k(logits, 8)   (sorted descending)
  weights = softmax(values)
Returns (weights f32 [262144, 8], indices int32 [262144, 8]).

Sharding: tokens split evenly across 8 cores (row-parallel, no comms).
Per-core layout: tokens on SBUF partitions (128 at a time), experts on the
free axis; DVE InstMax/InstMaxIndex produce the sorted top-8 directly.
"""

import sys

for _p in ("/opt/trn_rl_repo",):
    if _p not in sys.path:
        sys.path.insert(0, _p)

from contextlib import ExitStack

import numpy as np

import concourse.bacc as bacc
import concourse.mybir as mybir
import concourse.tile as tile
from concourse.bass_utils import run_bass_kernel_spmd

N_CORES = 8
T_FULL = 262144          # total tokens
E = 128                  # experts
K = 8                    # experts per token
T = T_FULL // N_CORES    # tokens per core (32768)
P = 128                  # tokens per DVE instruction (SBUF partitions)
TILES_PER_GROUP = 16     # 128-token tiles per DMA group
GROUP_T = P * TILES_PER_GROUP          # 2048 tokens per group
N_GROUPS = T // GROUP_T                # 16 groups per core

DEFAULT_VARIANT = "leanb"

_cached = {}


def _build_nc(variant, reps=1):
    """variant:
      full        - max+max_index+softmax (exp on ACT; reduce/recip/mul on DVE)
      offload     - exp+sum fused on ACT per tile, divide on GpSimd
                    (normalize_recip); DVE only does max/max_index
      batched     - offload + output stores batched over 4 groups (fewer
                    HWDGE lane conflicts between loads and stores) + deeper
                    load prefetch
      topk_only   - max+max_index, weights output = raw top-8 values
      max_only    - max only, indices output never written
    """
    if variant == "leanb":
        return _build_leanb(reps)
    if variant in ("lean", "lean32", "leanpg", "exponly", "nodiv",
                   "leanmul", "leanactdiv", "leanmulpg", "leanactdivpg"):
        return _build_lean(
            reps, tpg=(32 if variant == "lean32" else TILES_PER_GROUP),
            store_batch=(1 if variant.endswith("pg") or variant in
                         ("exponly", "nodiv") else STORE_BATCH),
            softmax=("none" if variant == "exponly"
                     else "nodiv" if variant == "nodiv"
                     else "poolmul" if variant.startswith("leanmul")
                     else "actdiv" if variant.startswith("leanactdiv")
                     else "full"))
    if variant in ("batched", "actdiv", "batched32"):
        return _build_batched(
            reps,
            divide_on=("act" if variant == "actdiv" else "gpsimd"),
            tpg=(32 if variant == "batched32" else TILES_PER_GROUP))
    nc = bacc.Bacc("TRN2", target_bir_lowering=False, debug=False,
                   enable_asserts=False)
    x = nc.dram_tensor("x", [T, E], mybir.dt.float32, kind="ExternalInput")
    w = nc.dram_tensor("w", [T, K], mybir.dt.float32, kind="ExternalOutput")
    ind = nc.dram_tensor("ind", [T, K], mybir.dt.uint32, kind="ExternalOutput")

    x_ap = x.ap()
    w_ap = w.ap()
    i_ap = ind.ap()

    with tile.TileContext(nc) as tc, ExitStack() as ctx:
        # Max/MaxIndex lower to the DVE BN-stats ISA struct which cannot
        # carry extra sync waits — give every pool whose first accessor is a
        # Max/MaxIndex one slot per group so no WAR waits are ever needed.
        xpool = ctx.enter_context(tc.tile_pool(name="x", bufs=3))
        vpool = ctx.enter_context(tc.tile_pool(name="v", bufs=N_GROUPS))
        ipool = ctx.enter_context(tc.tile_pool(name="i", bufs=N_GROUPS))
        epool = ctx.enter_context(tc.tile_pool(name="e", bufs=N_GROUPS))
        spool = ctx.enter_context(tc.tile_pool(name="s", bufs=N_GROUPS))
        wpool = ctx.enter_context(tc.tile_pool(name="w", bufs=N_GROUPS))

        for rep in range(reps):
          for g in range(N_GROUPS):
            lo, hi = g * GROUP_T, (g + 1) * GROUP_T
            # token t of this group lives at partition (t // 16), column
            # (t % 16): DRAM-contiguous 8KB per partition on the load, and
            # 512B-contiguous runs on both output stores.
            xt = xpool.tile([P, TILES_PER_GROUP, E], mybir.dt.float32)
            nc.sync.dma_start(
                xt[:], x_ap[lo:hi, :].rearrange("(p c) e -> p c e", p=P))

            vt = vpool.tile([P, TILES_PER_GROUP, K], mybir.dt.float32)
            it = ipool.tile([P, TILES_PER_GROUP, K], mybir.dt.uint32)
            for c in range(TILES_PER_GROUP):
                nc.vector.max(vt[:, c, :], xt[:, c, :])
                if variant != "max_only":
                    nc.vector.max_index(it[:, c, :], vt[:, c, :], xt[:, c, :])

            if variant in ("topk_only", "max_only"):
                nc.sync.dma_start(
                    w_ap[lo:hi, :].rearrange("(p c) k -> p c k", p=P), vt[:])
                if variant == "topk_only":
                    nc.sync.dma_start(
                        i_ap[lo:hi, :].rearrange("(p c) k -> p c k", p=P),
                        it[:])
                continue

            # softmax over the 8 selected logits; |logit| <= ~6 so exp() is
            # safe in f32 without subtracting the per-token max.
            et = epool.tile([P, TILES_PER_GROUP, K], mybir.dt.float32)
            st = spool.tile([P, TILES_PER_GROUP], mybir.dt.float32)
            wt = wpool.tile([P, TILES_PER_GROUP, K], mybir.dt.float32)
            if variant == "offload":
                for c in range(TILES_PER_GROUP):
                    nc.scalar.activation(
                        et[:, c, :], vt[:, c, :],
                        mybir.ActivationFunctionType.Exp,
                        accum_out=st[:, c:c + 1])
                for c in range(TILES_PER_GROUP):
                    nc.gpsimd.normalize_recip(
                        wt[:, c, :], et[:, c, :], st[:, c:c + 1])
            else:
                nc.scalar.activation(et[:], vt[:],
                                     mybir.ActivationFunctionType.Exp)
                nc.vector.reduce_sum(st[:], et[:], axis=mybir.AxisListType.X)
                rt = spool.tile([P, TILES_PER_GROUP], mybir.dt.float32)
                nc.vector.reciprocal(rt[:], st[:])
                nc.vector.tensor_mul(
                    wt[:], et[:],
                    rt[:].unsqueeze(2).broadcast_to([P, TILES_PER_GROUP, K]))

            nc.sync.dma_start(
                w_ap[lo:hi, :].rearrange("(p c) k -> p c k", p=P), wt[:])
            nc.sync.dma_start(
                i_ap[lo:hi, :].rearrange("(p c) k -> p c k", p=P), it[:])
    nc.compile()
    return nc


STORE_BATCH = 4                        # groups per output store DMA
N_BATCHES = N_GROUPS // STORE_BATCH


def _build_lean(reps=1, tpg=TILES_PER_GROUP, store_batch=STORE_BATCH,
                softmax="full"):
    """DVE: max/max_index + ONE reduce_sum per group (+194ns). ACT: ONE exp
    per group (FD=tpg*K) instead of 16 exp+accum (~6us/group): keeps ACT at
    ~5% busy so DVE is the only near-critical engine. Pool: per-tile
    normalize_recip divide (benign per actdiv-vs-batched comparison).
    """
    TILES_PER_GROUP = tpg
    GROUP_T = P * TILES_PER_GROUP
    N_GROUPS = T // GROUP_T
    STORE_BATCH = store_batch
    N_BATCHES = N_GROUPS // STORE_BATCH
    nc = bacc.Bacc("TRN2", target_bir_lowering=False, debug=False,
                   enable_asserts=False)
    x = nc.dram_tensor("x", [T, E], mybir.dt.float32, kind="ExternalInput")
    w = nc.dram_tensor("w", [T, K], mybir.dt.float32, kind="ExternalOutput")
    ind = nc.dram_tensor("ind", [T, K], mybir.dt.uint32, kind="ExternalOutput")

    x_ap = x.ap()
    w_ap = w.ap()
    i_ap = ind.ap()
    BT = STORE_BATCH * GROUP_T

    with tile.TileContext(nc) as tc, ExitStack() as ctx:
        xpool = ctx.enter_context(tc.tile_pool(name="x", bufs=5))
        vpool = ctx.enter_context(tc.tile_pool(name="v", bufs=N_GROUPS))
        epool = ctx.enter_context(tc.tile_pool(name="e", bufs=N_GROUPS))
        spool = ctx.enter_context(tc.tile_pool(name="s", bufs=N_GROUPS))
        ipool = ctx.enter_context(tc.tile_pool(name="i", bufs=N_BATCHES))
        wpool = ctx.enter_context(tc.tile_pool(name="w", bufs=N_BATCHES))

        FK = TILES_PER_GROUP * K

        for rep in range(reps):
            for b in range(N_BATCHES):
                it = ipool.tile([P, STORE_BATCH, TILES_PER_GROUP, K],
                                mybir.dt.uint32)
                wt = wpool.tile([P, STORE_BATCH, TILES_PER_GROUP, K],
                                mybir.dt.float32)
                for gb in range(STORE_BATCH):
                    g = b * STORE_BATCH + gb
                    lo, hi = g * GROUP_T, (g + 1) * GROUP_T
                    xt = xpool.tile([P, TILES_PER_GROUP, E], mybir.dt.float32)
                    nc.sync.dma_start(
                        xt[:],
                        x_ap[lo:hi, :].rearrange("(p c) e -> p c e", p=P))

                    vt = vpool.tile([P, FK], mybir.dt.float32)
                    for c in range(TILES_PER_GROUP):
                        nc.vector.max(vt[:, c * K:(c + 1) * K], xt[:, c, :])
                    for c in range(TILES_PER_GROUP):
                        nc.vector.max_index(it[:, gb, c, :],
                                            vt[:, c * K:(c + 1) * K],
                                            xt[:, c, :])

                    et = epool.tile([P, FK], mybir.dt.float32)
                    nc.scalar.activation(et[:], vt[:],
                                         mybir.ActivationFunctionType.Exp)
                    if softmax == "none":
                        nc.scalar.copy(
                            wt[:, gb].rearrange("p c k -> p (c k)"), et[:])
                        continue
                    st = spool.tile([P, TILES_PER_GROUP], mybir.dt.float32)
                    nc.vector.reduce_sum(
                        st[:], et[:].rearrange("p (c k) -> p c k", k=K),
                        axis=mybir.AxisListType.X)
                    if softmax == "nodiv":
                        nc.scalar.copy(
                            wt[:, gb].rearrange("p c k -> p (c k)"), et[:])
                        continue
                    if softmax == "full":
                        for c in range(TILES_PER_GROUP):
                            nc.gpsimd.normalize_recip(
                                wt[:, gb, c, :], et[:, c * K:(c + 1) * K],
                                st[:, c:c + 1])
                        continue
                    rt = spool.tile([P, TILES_PER_GROUP], mybir.dt.float32)
                    nc.vector.reciprocal(rt[:], st[:])
                    if softmax == "poolmul":
                        # one Pool op per group: wt = et * (1/sum) broadcast
                        nc.gpsimd.tensor_mul(
                            wt[:, gb],
                            et[:].rearrange("p (c k) -> p c k", k=K),
                            rt[:].unsqueeze(2).broadcast_to(
                                [P, TILES_PER_GROUP, K]))
                    else:  # actdiv: per-tile ACT copy with per-partition scale
                        for c in range(TILES_PER_GROUP):
                            nc.scalar.activation(
                                wt[:, gb, c, :], et[:, c * K:(c + 1) * K],
                                mybir.ActivationFunctionType.Copy,
                                scale=rt[:, c:c + 1])

                blo, bhi = b * BT, (b + 1) * BT
                nc.sync.dma_start(
                    w_ap[blo:bhi, :].rearrange(
                        "(g p c) k -> p g c k", g=STORE_BATCH, p=P), wt[:])
                nc.sync.dma_start(
                    i_ap[blo:bhi, :].rearrange(
                        "(g p c) k -> p g c k", g=STORE_BATCH, p=P), it[:])
    nc.compile()
    return nc


def _build_leanb(reps=1):
    """lean with per-BATCH exp and reduce_sum (FD=512 instead of 4x FD=128):
    amortizes the DVE 58-cycle and ACT 352-cycle fixed overheads; DVE drops
    from 16 to 4 reduce_sum instructions per rep (-696 cycles)."""
    nc = bacc.Bacc("TRN2", target_bir_lowering=False, debug=False,
                   enable_asserts=False)
    x = nc.dram_tensor("x", [T, E], mybir.dt.float32, kind="ExternalInput")
    w = nc.dram_tensor("w", [T, K], mybir.dt.float32, kind="ExternalOutput")
    ind = nc.dram_tensor("ind", [T, K], mybir.dt.uint32, kind="ExternalOutput")

    x_ap = x.ap()
    w_ap = w.ap()
    i_ap = ind.ap()
    BT = STORE_BATCH * GROUP_T
    TPB = STORE_BATCH * TILES_PER_GROUP        # tiles per batch (64)
    FKB = TPB * K                              # exp/reduce free size (512)

    with tile.TileContext(nc) as tc, ExitStack() as ctx:
        xpool = ctx.enter_context(tc.tile_pool(name="x", bufs=5))
        vpool = ctx.enter_context(tc.tile_pool(name="v", bufs=N_BATCHES))
        epool = ctx.enter_context(tc.tile_pool(name="e", bufs=N_BATCHES))
        spool = ctx.enter_context(tc.tile_pool(name="s", bufs=N_BATCHES))
        ipool = ctx.enter_context(tc.tile_pool(name="i", bufs=N_BATCHES))
        wpool = ctx.enter_context(tc.tile_pool(name="w", bufs=N_BATCHES))

        for rep in range(reps):
            for b in range(N_BATCHES):
                it = ipool.tile([P, STORE_BATCH, TILES_PER_GROUP, K],
                                mybir.dt.uint32)
                wt = wpool.tile([P, STORE_BATCH, TILES_PER_GROUP, K],
                                mybir.dt.float32)
                vt = vpool.tile([P, FKB], mybir.dt.float32)
                for gb in range(STORE_BATCH):
                    g = b * STORE_BATCH + gb
                    lo, hi = g * GROUP_T, (g + 1) * GROUP_T
                    xt = xpool.tile([P, TILES_PER_GROUP, E], mybir.dt.float32)
                    nc.sync.dma_start(
                        xt[:],
                        x_ap[lo:hi, :].rearrange("(p c) e -> p c e", p=P))
                    o = gb * TILES_PER_GROUP * K
                    for c in range(TILES_PER_GROUP):
                        nc.vector.max(vt[:, o + c * K:o + (c + 1) * K],
                                      xt[:, c, :])
                    for c in range(TILES_PER_GROUP):
                        nc.vector.max_index(it[:, gb, c, :],
                                            vt[:, o + c * K:o + (c + 1) * K],
                                            xt[:, c, :])

                et = epool.tile([P, FKB], mybir.dt.float32)
                nc.scalar.activation(et[:], vt[:],
                                     mybir.ActivationFunctionType.Exp)
                st = spool.tile([P, TPB], mybir.dt.float32)
                nc.vector.reduce_sum(
                    st[:], et[:].rearrange("p (t k) -> p t k", k=K),
                    axis=mybir.AxisListType.X)
                for gb in range(STORE_BATCH):
                    o = gb * TILES_PER_GROUP
                    for c in range(TILES_PER_GROUP):
                        t = o + c
                        nc.gpsimd.normalize_recip(
                            wt[:, gb, c, :], et[:, t * K:(t + 1) * K],
                            st[:, t:t + 1])

                blo, bhi = b * BT, (b + 1) * BT
                nc.sync.dma_start(
                    w_ap[blo:bhi, :].rearrange(
                        "(g p c) k -> p g c k", g=STORE_BATCH, p=P), wt[:])
                nc.sync.dma_start(
                    i_ap[blo:bhi, :].rearrange(
                        "(g p c) k -> p g c k", g=STORE_BATCH, p=P), it[:])
    nc.compile()
    return nc


def _build_batched(reps=1, divide_on="gpsimd", tpg=TILES_PER_GROUP):
    TILES_PER_GROUP = tpg
    GROUP_T = P * TILES_PER_GROUP
    N_GROUPS = T // GROUP_T
    N_BATCHES = N_GROUPS // STORE_BATCH
    nc = bacc.Bacc("TRN2", target_bir_lowering=False, debug=False,
                   enable_asserts=False)
    x = nc.dram_tensor("x", [T, E], mybir.dt.float32, kind="ExternalInput")
    w = nc.dram_tensor("w", [T, K], mybir.dt.float32, kind="ExternalOutput")
    ind = nc.dram_tensor("ind", [T, K], mybir.dt.uint32, kind="ExternalOutput")

    x_ap = x.ap()
    w_ap = w.ap()
    i_ap = ind.ap()
    BT = STORE_BATCH * GROUP_T          # tokens per store batch (8192)

    with tile.TileContext(nc) as tc, ExitStack() as ctx:
        xpool = ctx.enter_context(tc.tile_pool(name="x", bufs=5))
        vpool = ctx.enter_context(tc.tile_pool(name="v", bufs=N_GROUPS))
        epool = ctx.enter_context(tc.tile_pool(name="e", bufs=N_GROUPS))
        spool = ctx.enter_context(tc.tile_pool(name="s", bufs=N_GROUPS))
        ipool = ctx.enter_context(tc.tile_pool(name="i", bufs=N_BATCHES))
        wpool = ctx.enter_context(tc.tile_pool(name="w", bufs=N_BATCHES))

        for rep in range(reps):
            for b in range(N_BATCHES):
                it = ipool.tile([P, STORE_BATCH, TILES_PER_GROUP, K],
                                mybir.dt.uint32)
                wt = wpool.tile([P, STORE_BATCH, TILES_PER_GROUP, K],
                                mybir.dt.float32)
                for gb in range(STORE_BATCH):
                    g = b * STORE_BATCH + gb
                    lo, hi = g * GROUP_T, (g + 1) * GROUP_T
                    xt = xpool.tile([P, TILES_PER_GROUP, E], mybir.dt.float32)
                    nc.sync.dma_start(
                        xt[:],
                        x_ap[lo:hi, :].rearrange("(p c) e -> p c e", p=P))

                    # all maxes first, then all max_indexes: puts ~16 instrs
                    # between the vt write and its same-engine readback so
                    # the BN unit never stalls on the SBUF write ack
                    vt = vpool.tile([P, TILES_PER_GROUP, K], mybir.dt.float32)
                    for c in range(TILES_PER_GROUP):
                        nc.vector.max(vt[:, c, :], xt[:, c, :])
                    for c in range(TILES_PER_GROUP):
                        nc.vector.max_index(it[:, gb, c, :], vt[:, c, :],
                                            xt[:, c, :])

                    et = epool.tile([P, TILES_PER_GROUP, K], mybir.dt.float32)
                    st = spool.tile([P, TILES_PER_GROUP], mybir.dt.float32)
                    if divide_on == "gpsimd":
                        for c in range(TILES_PER_GROUP):
                            nc.scalar.activation(
                                et[:, c, :], vt[:, c, :],
                                mybir.ActivationFunctionType.Exp,
                                accum_out=st[:, c:c + 1])
                        for c in range(TILES_PER_GROUP):
                            nc.gpsimd.normalize_recip(
                                wt[:, gb, c, :], et[:, c, :], st[:, c:c + 1])
                    else:
                        # keep GpSimd fully idle: its SBUF port is shared
                        # (exclusive lock) with the saturated DVE
                        nc.scalar.activation(
                            et[:], vt[:], mybir.ActivationFunctionType.Exp)
                        nc.vector.reduce_sum(st[:], et[:],
                                             axis=mybir.AxisListType.X)
                        rt = spool.tile([P, TILES_PER_GROUP],
                                        mybir.dt.float32)
                        nc.vector.reciprocal(rt[:], st[:])
                        for c in range(TILES_PER_GROUP):
                            nc.scalar.activation(
                                wt[:, gb, c, :], et[:, c, :],
                                mybir.ActivationFunctionType.Copy,
                                scale=rt[:, c:c + 1])

                blo, bhi = b * BT, (b + 1) * BT
                nc.sync.dma_start(
                    w_ap[blo:bhi, :].rearrange(
                        "(g p c) k -> p g c k", g=STORE_BATCH, p=P), wt[:])
                nc.sync.dma_start(
                    i_ap[blo:bhi, :].rearrange(
                        "(g p c) k -> p g c k", g=STORE_BATCH, p=P), it[:])
    nc.compile()
    return nc


def get_nc(variant=DEFAULT_VARIANT, reps=1):
    key = f"nc_{variant}_{reps}"
    if key not in _cached:
        _cached[key] = _build_nc(variant, reps)
    return _cached[key]


def run(gate_logits: np.ndarray, variant=DEFAULT_VARIANT, **spmd_kwargs):
    """Run the bass kernel on 8 cores; returns (weights, indices, results)."""
    gate_logits = np.ascontiguousarray(gate_logits, dtype=np.float32)
    assert gate_logits.shape == (T_FULL, E), gate_logits.shape
    nc = get_nc(variant)
    in_maps = [{"x": gate_logits[c * T:(c + 1) * T]} for c in range(N_CORES)]
    res = run_bass_kernel_spmd(nc, in_maps, core_ids=list(range(N_CORES)),
                               **spmd_kwargs)
    weights = np.concatenate([r["w"] for r in res.results], axis=0)
    indices = np.concatenate([r["ind"] for r in res.results], axis=0)
    return weights, indices.view(np.int32), res


def kernel(gate_logits: np.ndarray):
    weights, indices, _ = run(gate_logits)
    return weights, indices



# revision 12
# speedup vs baseline: 1.2599x; 1.2599x over previous
"""MoE top-8 routing kernel for Trainium2 (8 NeuronCores, data-parallel).

Computes, for each of 262144 tokens with 128 expert logits:
  values, indices = top_k(logits, 8)   (sorted descending)
  weights = softmax(values)
Returns (weights f32 [262144, 8], indices int32 [262144, 8]).

Sharding: tokens split evenly across 8 cores (row-parallel, no comms).
Per-core layout: tokens on SBUF partitions (128 at a time), experts on the
free axis; DVE InstMax/InstMaxIndex produce the sorted top-8 directly.
"""

import sys

for _p in ("/opt/trn_rl_repo",):
    if _p not in sys.path:
        sys.path.insert(0, _p)

from contextlib import ExitStack

import numpy as np

import concourse.bacc as bacc
import concourse.mybir as mybir
import concourse.tile as tile
from concourse.bass_utils import run_bass_kernel_spmd

N_CORES = 8
T_FULL = 262144          # total tokens
E = 128                  # experts
K = 8                    # experts per token
T = T_FULL // N_CORES    # tokens per core (32768)
P = 128                  # tokens per DVE instruction (SBUF partitions)
TILES_PER_GROUP = 16     # 128-token tiles per DMA group
GROUP_T = P * TILES_PER_GROUP          # 2048 tokens per group
N_GROUPS = T // GROUP_T                # 16 groups per core

DEFAULT_VARIANT = "leanb"

_cached = {}


def _build_nc(variant, reps=1):
    """variant:
      full        - max+max_index+softmax (exp on ACT; reduce/recip/mul on DVE)
      offload     - exp+sum fused on ACT per tile, divide on GpSimd
                    (normalize_recip); DVE only does max/max_index
      batched     - offload + output stores batched over 4 groups (fewer
                    HWDGE lane conflicts between loads and stores) + deeper
                    load prefetch
      topk_only   - max+max_index, weights output = raw top-8 values
      max_only    - max only, indices output never written
    """
    if variant in ("leanb", "leanb8"):
        return _build_leanb(reps, store_batch=(8 if variant == "leanb8"
                                               else STORE_BATCH))
    if variant in ("lean", "lean32", "leanpg", "exponly", "nodiv",
                   "leanmul", "leanactdiv", "leanmulpg", "leanactdivpg"):
        return _build_lean(
            reps, tpg=(32 if variant == "lean32" else TILES_PER_GROUP),
            store_batch=(1 if variant.endswith("pg") or variant in
                         ("exponly", "nodiv") else STORE_BATCH),
            softmax=("none" if variant == "exponly"
                     else "nodiv" if variant == "nodiv"
                     else "poolmul" if variant.startswith("leanmul")
                     else "actdiv" if variant.startswith("leanactdiv")
                     else "full"))
    if variant in ("batched", "actdiv", "batched32"):
        return _build_batched(
            reps,
            divide_on=("act" if variant == "actdiv" else "gpsimd"),
            tpg=(32 if variant == "batched32" else TILES_PER_GROUP))
    nc = bacc.Bacc("TRN2", target_bir_lowering=False, debug=False,
                   enable_asserts=False)
    x = nc.dram_tensor("x", [T, E], mybir.dt.float32, kind="ExternalInput")
    w = nc.dram_tensor("w", [T, K], mybir.dt.float32, kind="ExternalOutput")
    ind = nc.dram_tensor("ind", [T, K], mybir.dt.uint32, kind="ExternalOutput")

    x_ap = x.ap()
    w_ap = w.ap()
    i_ap = ind.ap()

    with tile.TileContext(nc) as tc, ExitStack() as ctx:
        # Max/MaxIndex lower to the DVE BN-stats ISA struct which cannot
        # carry extra sync waits — give every pool whose first accessor is a
        # Max/MaxIndex one slot per group so no WAR waits are ever needed.
        xpool = ctx.enter_context(tc.tile_pool(name="x", bufs=3))
        vpool = ctx.enter_context(tc.tile_pool(name="v", bufs=N_GROUPS))
        ipool = ctx.enter_context(tc.tile_pool(name="i", bufs=N_GROUPS))
        epool = ctx.enter_context(tc.tile_pool(name="e", bufs=N_GROUPS))
        spool = ctx.enter_context(tc.tile_pool(name="s", bufs=N_GROUPS))
        wpool = ctx.enter_context(tc.tile_pool(name="w", bufs=N_GROUPS))

        for rep in range(reps):
          for g in range(N_GROUPS):
            lo, hi = g * GROUP_T, (g + 1) * GROUP_T
            # token t of this group lives at partition (t // 16), column
            # (t % 16): DRAM-contiguous 8KB per partition on the load, and
            # 512B-contiguous runs on both output stores.
            xt = xpool.tile([P, TILES_PER_GROUP, E], mybir.dt.float32)
            nc.sync.dma_start(
                xt[:], x_ap[lo:hi, :].rearrange("(p c) e -> p c e", p=P))

            vt = vpool.tile([P, TILES_PER_GROUP, K], mybir.dt.float32)
            it = ipool.tile([P, TILES_PER_GROUP, K], mybir.dt.uint32)
            for c in range(TILES_PER_GROUP):
                nc.vector.max(vt[:, c, :], xt[:, c, :])
                if variant != "max_only":
                    nc.vector.max_index(it[:, c, :], vt[:, c, :], xt[:, c, :])

            if variant in ("topk_only", "max_only"):
                nc.sync.dma_start(
                    w_ap[lo:hi, :].rearrange("(p c) k -> p c k", p=P), vt[:])
                if variant == "topk_only":
                    nc.sync.dma_start(
                        i_ap[lo:hi, :].rearrange("(p c) k -> p c k", p=P),
                        it[:])
                continue

            # softmax over the 8 selected logits; |logit| <= ~6 so exp() is
            # safe in f32 without subtracting the per-token max.
            et = epool.tile([P, TILES_PER_GROUP, K], mybir.dt.float32)
            st = spool.tile([P, TILES_PER_GROUP], mybir.dt.float32)
            wt = wpool.tile([P, TILES_PER_GROUP, K], mybir.dt.float32)
            if variant == "offload":
                for c in range(TILES_PER_GROUP):
                    nc.scalar.activation(
                        et[:, c, :], vt[:, c, :],
                        mybir.ActivationFunctionType.Exp,
                        accum_out=st[:, c:c + 1])
                for c in range(TILES_PER_GROUP):
                    nc.gpsimd.normalize_recip(
                        wt[:, c, :], et[:, c, :], st[:, c:c + 1])
            else:
                nc.scalar.activation(et[:], vt[:],
                                     mybir.ActivationFunctionType.Exp)
                nc.vector.reduce_sum(st[:], et[:], axis=mybir.AxisListType.X)
                rt = spool.tile([P, TILES_PER_GROUP], mybir.dt.float32)
                nc.vector.reciprocal(rt[:], st[:])
                nc.vector.tensor_mul(
                    wt[:], et[:],
                    rt[:].unsqueeze(2).broadcast_to([P, TILES_PER_GROUP, K]))

            nc.sync.dma_start(
                w_ap[lo:hi, :].rearrange("(p c) k -> p c k", p=P), wt[:])
            nc.sync.dma_start(
                i_ap[lo:hi, :].rearrange("(p c) k -> p c k", p=P), it[:])
    nc.compile()
    return nc


STORE_BATCH = 4                        # groups per output store DMA
N_BATCHES = N_GROUPS // STORE_BATCH


def _build_lean(reps=1, tpg=TILES_PER_GROUP, store_batch=STORE_BATCH,
                softmax="full"):
    """DVE: max/max_index + ONE reduce_sum per group (+194ns). ACT: ONE exp
    per group (FD=tpg*K) instead of 16 exp+accum (~6us/group): keeps ACT at
    ~5% busy so DVE is the only near-critical engine. Pool: per-tile
    normalize_recip divide (benign per actdiv-vs-batched comparison).
    """
    TILES_PER_GROUP = tpg
    GROUP_T = P * TILES_PER_GROUP
    N_GROUPS = T // GROUP_T
    STORE_BATCH = store_batch
    N_BATCHES = N_GROUPS // STORE_BATCH
    nc = bacc.Bacc("TRN2", target_bir_lowering=False, debug=False,
                   enable_asserts=False)
    x = nc.dram_tensor("x", [T, E], mybir.dt.float32, kind="ExternalInput")
    w = nc.dram_tensor("w", [T, K], mybir.dt.float32, kind="ExternalOutput")
    ind = nc.dram_tensor("ind", [T, K], mybir.dt.uint32, kind="ExternalOutput")

    x_ap = x.ap()
    w_ap = w.ap()
    i_ap = ind.ap()
    BT = STORE_BATCH * GROUP_T

    with tile.TileContext(nc) as tc, ExitStack() as ctx:
        xpool = ctx.enter_context(tc.tile_pool(name="x", bufs=5))
        vpool = ctx.enter_context(tc.tile_pool(name="v", bufs=N_GROUPS))
        epool = ctx.enter_context(tc.tile_pool(name="e", bufs=N_GROUPS))
        spool = ctx.enter_context(tc.tile_pool(name="s", bufs=N_GROUPS))
        ipool = ctx.enter_context(tc.tile_pool(name="i", bufs=N_BATCHES))
        wpool = ctx.enter_context(tc.tile_pool(name="w", bufs=N_BATCHES))

        FK = TILES_PER_GROUP * K

        for rep in range(reps):
            for b in range(N_BATCHES):
                it = ipool.tile([P, STORE_BATCH, TILES_PER_GROUP, K],
                                mybir.dt.uint32)
                wt = wpool.tile([P, STORE_BATCH, TILES_PER_GROUP, K],
                                mybir.dt.float32)
                for gb in range(STORE_BATCH):
                    g = b * STORE_BATCH + gb
                    lo, hi = g * GROUP_T, (g + 1) * GROUP_T
                    xt = xpool.tile([P, TILES_PER_GROUP, E], mybir.dt.float32)
                    nc.sync.dma_start(
                        xt[:],
                        x_ap[lo:hi, :].rearrange("(p c) e -> p c e", p=P))

                    vt = vpool.tile([P, FK], mybir.dt.float32)
                    for c in range(TILES_PER_GROUP):
                        nc.vector.max(vt[:, c * K:(c + 1) * K], xt[:, c, :])
                    for c in range(TILES_PER_GROUP):
                        nc.vector.max_index(it[:, gb, c, :],
                                            vt[:, c * K:(c + 1) * K],
                                            xt[:, c, :])

                    et = epool.tile([P, FK], mybir.dt.float32)
                    nc.scalar.activation(et[:], vt[:],
                                         mybir.ActivationFunctionType.Exp)
                    if softmax == "none":
                        nc.scalar.copy(
                            wt[:, gb].rearrange("p c k -> p (c k)"), et[:])
                        continue
                    st = spool.tile([P, TILES_PER_GROUP], mybir.dt.float32)
                    nc.vector.reduce_sum(
                        st[:], et[:].rearrange("p (c k) -> p c k", k=K),
                        axis=mybir.AxisListType.X)
                    if softmax == "nodiv":
                        nc.scalar.copy(
                            wt[:, gb].rearrange("p c k -> p (c k)"), et[:])
                        continue
                    if softmax == "full":
                        for c in range(TILES_PER_GROUP):
                            nc.gpsimd.normalize_recip(
                                wt[:, gb, c, :], et[:, c * K:(c + 1) * K],
                                st[:, c:c + 1])
                        continue
                    rt = spool.tile([P, TILES_PER_GROUP], mybir.dt.float32)
                    nc.vector.reciprocal(rt[:], st[:])
                    if softmax == "poolmul":
                        # one Pool op per group: wt = et * (1/sum) broadcast
                        nc.gpsimd.tensor_mul(
                            wt[:, gb],
                            et[:].rearrange("p (c k) -> p c k", k=K),
                            rt[:].unsqueeze(2).broadcast_to(
                                [P, TILES_PER_GROUP, K]))
                    else:  # actdiv: per-tile ACT copy with per-partition scale
                        for c in range(TILES_PER_GROUP):
                            nc.scalar.activation(
                                wt[:, gb, c, :], et[:, c * K:(c + 1) * K],
                                mybir.ActivationFunctionType.Copy,
                                scale=rt[:, c:c + 1])

                blo, bhi = b * BT, (b + 1) * BT
                nc.sync.dma_start(
                    w_ap[blo:bhi, :].rearrange(
                        "(g p c) k -> p g c k", g=STORE_BATCH, p=P), wt[:])
                nc.sync.dma_start(
                    i_ap[blo:bhi, :].rearrange(
                        "(g p c) k -> p g c k", g=STORE_BATCH, p=P), it[:])
    nc.compile()
    return nc


def _build_leanb(reps=1, store_batch=STORE_BATCH):
    """lean with per-BATCH exp and reduce_sum (FD=512 instead of 4x FD=128):
    amortizes the DVE 58-cycle and ACT 352-cycle fixed overheads; DVE drops
    from 16 to 4 reduce_sum instructions per rep (-696 cycles)."""
    nc = bacc.Bacc("TRN2", target_bir_lowering=False, debug=False,
                   enable_asserts=False)
    x = nc.dram_tensor("x", [T, E], mybir.dt.float32, kind="ExternalInput")
    w = nc.dram_tensor("w", [T, K], mybir.dt.float32, kind="ExternalOutput")
    ind = nc.dram_tensor("ind", [T, K], mybir.dt.uint32, kind="ExternalOutput")

    x_ap = x.ap()
    w_ap = w.ap()
    i_ap = ind.ap()
    STORE_BATCH = store_batch
    N_BATCHES = N_GROUPS // STORE_BATCH
    BT = STORE_BATCH * GROUP_T
    TPB = STORE_BATCH * TILES_PER_GROUP        # tiles per batch (64)
    FKB = TPB * K                              # exp/reduce free size (512)

    with tile.TileContext(nc) as tc, ExitStack() as ctx:
        xpool = ctx.enter_context(tc.tile_pool(name="x", bufs=5))
        vpool = ctx.enter_context(tc.tile_pool(name="v", bufs=N_BATCHES))
        epool = ctx.enter_context(tc.tile_pool(name="e", bufs=N_BATCHES))
        spool = ctx.enter_context(tc.tile_pool(name="s", bufs=N_BATCHES))
        ipool = ctx.enter_context(tc.tile_pool(name="i", bufs=N_BATCHES))
        wpool = ctx.enter_context(tc.tile_pool(name="w", bufs=N_BATCHES))

        for rep in range(reps):
            for b in range(N_BATCHES):
                it = ipool.tile([P, STORE_BATCH, TILES_PER_GROUP, K],
                                mybir.dt.uint32)
                wt = wpool.tile([P, STORE_BATCH, TILES_PER_GROUP, K],
                                mybir.dt.float32)
                vt = vpool.tile([P, FKB], mybir.dt.float32)
                for gb in range(STORE_BATCH):
                    g = b * STORE_BATCH + gb
                    lo, hi = g * GROUP_T, (g + 1) * GROUP_T
                    xt = xpool.tile([P, TILES_PER_GROUP, E], mybir.dt.float32)
                    nc.sync.dma_start(
                        xt[:],
                        x_ap[lo:hi, :].rearrange("(p c) e -> p c e", p=P))
                    o = gb * TILES_PER_GROUP * K
                    for c in range(TILES_PER_GROUP):
                        nc.vector.max(vt[:, o + c * K:o + (c + 1) * K],
                                      xt[:, c, :])
                    for c in range(TILES_PER_GROUP):
                        nc.vector.max_index(it[:, gb, c, :],
                                            vt[:, o + c * K:o + (c + 1) * K],
                                            xt[:, c, :])

                et = epool.tile([P, FKB], mybir.dt.float32)
                nc.scalar.activation(et[:], vt[:],
                                     mybir.ActivationFunctionType.Exp)
                st = spool.tile([P, TPB], mybir.dt.float32)
                nc.vector.reduce_sum(
                    st[:], et[:].rearrange("p (t k) -> p t k", k=K),
                    axis=mybir.AxisListType.X)
                for gb in range(STORE_BATCH):
                    o = gb * TILES_PER_GROUP
                    for c in range(TILES_PER_GROUP):
                        t = o + c
                        nc.gpsimd.normalize_recip(
                            wt[:, gb, c, :], et[:, t * K:(t + 1) * K],
                            st[:, t:t + 1])

                blo, bhi = b * BT, (b + 1) * BT
                nc.sync.dma_start(
                    w_ap[blo:bhi, :].rearrange(
                        "(g p c) k -> p g c k", g=STORE_BATCH, p=P), wt[:])
                nc.sync.dma_start(
                    i_ap[blo:bhi, :].rearrange(
                        "(g p c) k -> p g c k", g=STORE_BATCH, p=P), it[:])
    nc.compile()
    return nc


def _build_batched(reps=1, divide_on="gpsimd", tpg=TILES_PER_GROUP):
    TILES_PER_GROUP = tpg
    GROUP_T = P * TILES_PER_GROUP
    N_GROUPS = T // GROUP_T
    N_BATCHES = N_GROUPS // STORE_BATCH
    nc = bacc.Bacc("TRN2", target_bir_lowering=False, debug=False,
                   enable_asserts=False)
    x = nc.dram_tensor("x", [T, E], mybir.dt.float32, kind="ExternalInput")
    w = nc.dram_tensor("w", [T, K], mybir.dt.float32, kind="ExternalOutput")
    ind = nc.dram_tensor("ind", [T, K], mybir.dt.uint32, kind="ExternalOutput")

    x_ap = x.ap()
    w_ap = w.ap()
    i_ap = ind.ap()
    BT = STORE_BATCH * GROUP_T          # tokens per store batch (8192)

    with tile.TileContext(nc) as tc, ExitStack() as ctx:
        xpool = ctx.enter_context(tc.tile_pool(name="x", bufs=5))
        vpool = ctx.enter_context(tc.tile_pool(name="v", bufs=N_GROUPS))
        epool = ctx.enter_context(tc.tile_pool(name="e", bufs=N_GROUPS))
        spool = ctx.enter_context(tc.tile_pool(name="s", bufs=N_GROUPS))
        ipool = ctx.enter_context(tc.tile_pool(name="i", bufs=N_BATCHES))
        wpool = ctx.enter_context(tc.tile_pool(name="w", bufs=N_BATCHES))

        for rep in range(reps):
            for b in range(N_BATCHES):
                it = ipool.tile([P, STORE_BATCH, TILES_PER_GROUP, K],
                                mybir.dt.uint32)
                wt = wpool.tile([P, STORE_BATCH, TILES_PER_GROUP, K],
                                mybir.dt.float32)
                for gb in range(STORE_BATCH):
                    g = b * STORE_BATCH + gb
                    lo, hi = g * GROUP_T, (g + 1) * GROUP_T
                    xt = xpool.tile([P, TILES_PER_GROUP, E], mybir.dt.float32)
                    nc.sync.dma_start(
                        xt[:],
                        x_ap[lo:hi, :].rearrange("(p c) e -> p c e", p=P))

                    # all maxes first, then all max_indexes: puts ~16 instrs
                    # between the vt write and its same-engine readback so
                    # the BN unit never stalls on the SBUF write ack
                    vt = vpool.tile([P, TILES_PER_GROUP, K], mybir.dt.float32)
                    for c in range(TILES_PER_GROUP):
                        nc.vector.max(vt[:, c, :], xt[:, c, :])
                    for c in range(TILES_PER_GROUP):
                        nc.vector.max_index(it[:, gb, c, :], vt[:, c, :],
                                            xt[:, c, :])

                    et = epool.tile([P, TILES_PER_GROUP, K], mybir.dt.float32)
                    st = spool.tile([P, TILES_PER_GROUP], mybir.dt.float32)
                    if divide_on == "gpsimd":
                        for c in range(TILES_PER_GROUP):
                            nc.scalar.activation(
                                et[:, c, :], vt[:, c, :],
                                mybir.ActivationFunctionType.Exp,
                                accum_out=st[:, c:c + 1])
                        for c in range(TILES_PER_GROUP):
                            nc.gpsimd.normalize_recip(
                                wt[:, gb, c, :], et[:, c, :], st[:, c:c + 1])
                    else:
                        # keep GpSimd fully idle: its SBUF port is shared
                        # (exclusive lock) with the saturated DVE
                        nc.scalar.activation(
                            et[:], vt[:], mybir.ActivationFunctionType.Exp)
                        nc.vector.reduce_sum(st[:], et[:],
                                             axis=mybir.AxisListType.X)
                        rt = spool.tile([P, TILES_PER_GROUP],
                                        mybir.dt.float32)
                        nc.vector.reciprocal(rt[:], st[:])
                        for c in range(TILES_PER_GROUP):
                            nc.scalar.activation(
                                wt[:, gb, c, :], et[:, c, :],
                                mybir.ActivationFunctionType.Copy,
                                scale=rt[:, c:c + 1])

                blo, bhi = b * BT, (b + 1) * BT
                nc.sync.dma_start(
                    w_ap[blo:bhi, :].rearrange(
                        "(g p c) k -> p g c k", g=STORE_BATCH, p=P), wt[:])
                nc.sync.dma_start(
                    i_ap[blo:bhi, :].rearrange(
                        "(g p c) k -> p g c k", g=STORE_BATCH, p=P), it[:])
    nc.compile()
    return nc


def get_nc(variant=DEFAULT_VARIANT, reps=1):
    key = f"nc_{variant}_{reps}"
    if key not in _cached:
        _cached[key] = _build_nc(variant, reps)
    return _cached[key]


def run(gate_logits: np.ndarray, variant=DEFAULT_VARIANT, **spmd_kwargs):
    """Run the bass kernel on 8 cores; returns (weights, indices, results)."""
    gate_logits = np.ascontiguousarray(gate_logits, dtype=np.float32)
    assert gate_logits.shape == (T_FULL, E), gate_logits.shape
    nc = get_nc(variant)
    in_maps = [{"x": gate_logits[c * T:(c + 1) * T]} for c in range(N_CORES)]
    res = run_bass_kernel_spmd(nc, in_maps, core_ids=list(range(N_CORES)),
                               **spmd_kwargs)
    weights = np.concatenate([r["w"] for r in res.results], axis=0)
    indices = np.concatenate([r["ind"] for r in res.results], axis=0)
    return weights, indices.view(np.int32), res


def kernel(gate_logits: np.ndarray):
    weights, indices, _ = run(gate_logits)
    return weights, indices

